# revision 1
# baseline (speedup 1.0000x reference)
"""Distributed top-k retrieval kernel (CoconutPPO) for 8 Trainium2 NeuronCores.

Strategy (per the standard distributed-ANN pattern):
 - The 500k x 256 memory bank (512 MB -- the memory-roofline term) is sharded
   row-wise across 8 cores. Each core streams its 62500-row strip through the
   TensorEngine against the (replicated) projected query block, producing
   weighted cosine sims, and reduces them on-chip to per-segment maxima
   (segments of 50 bank rows). Only [128, 1250] maxima per core return to the
   host: a 6400x reduction of the score matrix, so device traffic ~= one read
   of the bank.
 - Scores are scale-invariant per batch row, so the query is left unnormalized
   on-device; bank-row norms and memory-value weights are folded into the
   shipped bank strip on the host.
 - Host merges the 8x per-core segment maxima, picks top-C candidate segments
   per row, and exactly re-scores those few candidates in fp64 to recover the
   exact fp32 top-3 (validated: true top-3 segments rank <= 3rd of 10000, so
   C=16 has enormous margin even with a bf16 scan).
 - The tiny MLPs/heads (<< 1% of FLOPs; replicated weights per the hint) run
   on the host in fp32, bit-matching the reference.
"""

import base64
import os
import zlib

import numpy as np
import ml_dtypes

B, H, R, M, K = 128, 4096, 256, 500000, 3
N_CORES = 8
MS = M // N_CORES            # 62500 bank rows per core
SEG = 50                     # segment size for on-chip max-reduce
TILE_COLS = 2500             # columns per DMA tile ([128, 2500] = 1.25MB f32)
N_TILES = MS // TILE_COLS    # 25
CHUNK = 500                  # matmul moving free dim (1 PSUM bank, >=256 for f32r full rate)
N_SEG_CORE = MS // SEG       # 1250
C_SEGS = 16                  # candidate segments per row for exact host refine
FUSE = 0.5
EPS_NORM = 1e-12
EPS_VAL = 1e-8

SCAN = os.environ.get("COCO_SCAN_DTYPE", "bf16")  # "bf16" | "f32r"

# jax.random.gumbel(jax.random.key(1), (128, 2), float32) on CPU jax
# (threefry2x32, threefry_partitionable=True) -- fallback if jax is absent.
_GUMBEL_B64 = (
    "eNoNzos31mcAB3Bi5nLEXIZqb+q0HVtswmTl93wfRVbetCSsIqxxZq/mNjmLihJyyeWQRGUuuRSOW056n2ckmTjeHMklEiorhCPO"
    "XLfPX/Cp+KiJ9IaE8MHd/jTSMRwGOl3o8r+HZIcMblY0w7yri++3PG5AkZeYDI/lo1Bjr3RL8xAcfo+HB6ljfWOTOLI1CmaKfehI"
    "8be2HEjiq/5Lgu/zdqwqjhDzsp3cI7yXLfiaUj1PMTTVD/CLV02pW3QElJryoHJiUXpd3MNepYbT9FUTPhLjznjpVmxTSEPcpxZw"
    "WdXkbqo29CptgcvoMN5K7iD5xRj2nhPRgmPx5KWbAr4+rMmtbEP5uHYq1zw8ze7rd5Mbsw8wOt8H4+pJmCk38ZeZOgg9I4OF6UFU"
    "GIai7o8aPlA5LL2Y+FqoVhHBPFAJb9ViWOb7G2Qha4TEuQ5JL9/+ickcdHiUoy8i7N9gqa1ROP1FAvao1jOZ9gWqm+9HNKkzeiq/"
    "QfKDHpSYZgga1Y8w2/WOnJBmc1sDhgDlQH6+PxZOd5bZqHwhbXY5jZLWPv7kwld0JKmPhdnGItJDxuYkvhhY8IbakDvrNiT0mKsX"
    "5AK38zr57dRV7In5rDS2/KMGN3q3Bku/ZiGtS0KynJz54FprOjh0EPdU2oh+rj2djTuLDKUg+HW3C9E5adxyWoLoOVeSe1tE+eb1"
    "XLyijVmFFeJjulv6y+M8kt6gQ2U/vwH7ywp+6Slcq6UA3/uI+FhYGVQ6Y/nY4YfMXsVYuim8Bpsixbw8KIv98KSCTgZEweDuspCz"
    "fgmeRJdYGbUJip1V3OpzXdrcvgVph56xUlsJ/zA+gw/FATDpKGJeR5148OiUdId9KkJLUpB9N4gnn8xBu1ckpurqra81X0LQ/5fx"
    "6VuQBU1gl9+XMLJRFfzfPyRO/4i4XV44kRrE8NqurdxodYHsmpKh1+4m+XbPNnruURXM5OZIjLkJr0wxFsqVCpAg2Yez3qfgrj0s"
    "NK5MorcZ7FKvobTaVgtHgpPQKktgl7MLeIf7OjxTr4U44hSKtWz4yusrqDl0lK2URaJLeYwIr/bz3yyC0X6mBrmL5Vwk0kN3gQ+v"
    "2hDPHFLyIFKRg67fJzTfMgehWZ9xo+OsfkJxEsJOLSpvoQb1A3FIO95NHK/E4amOLTOMOQeNf+vRGRYCJckEz3dcIt7119HfW4qh"
    "+Qz8vbhqXTfsjP61G7j+xsT6FY8+UnRej0pb/4SdWz5f2LgX4TOTJDYzW6j1eIGByqcIOKFFG3ZcI8EnXXijgirO7FOmw8+/Y+v2"
    "D+Gm/hoq53CN689QnmhRxtcURsBdaT8cPr6F/wAWUe20"
)


def _gumbel_key1():
    """Gumbel noise matching jax.random.gumbel(key(1), (B, 2)) on CPU."""
    try:
        import jax
        import jax.numpy as jnp

        cpu = jax.devices("cpu")[0]
        with jax.default_device(cpu):
            return np.asarray(jax.random.gumbel(jax.random.key(1), (B, 2), jnp.float32))
    except Exception:
        return np.frombuffer(
            zlib.decompress(base64.b64decode(_GUMBEL_B64)), np.float32
        ).reshape(B, 2)


_module_cache = {}


def _get_module(scan):
    """Build (once) the SPMD Bass module for one core's bank-strip scan."""
    if scan in _module_cache:
        return _module_cache[scan]

    import concourse.bacc as bacc
    import concourse.tile as tile
    import concourse.mybir as mybir

    dt_scan = mybir.dt.bfloat16 if scan == "bf16" else mybir.dt.float32r

    nc = bacc.Bacc(
        "TRN2",
        target_bir_lowering=False,
        debug=False,
        enable_asserts=False,
        num_devices=N_CORES,
    )
    rq = nc.dram_tensor("rq", (R, B), dt_scan, kind="ExternalInput").ap()
    bankT = nc.dram_tensor("bankT", (R, MS), dt_scan, kind="ExternalInput").ap()
    seg_dram = nc.dram_tensor(
        "segmax", (B, N_SEG_CORE), mybir.dt.float32, kind="ExternalOutput"
    ).ap()

    with tile.TileContext(nc) as tc:
        with (
            tc.tile_pool(name="const", bufs=1) as const_pool,
            tc.tile_pool(name="bank", bufs=4) as bank_pool,
            tc.tile_pool(name="outp", bufs=1) as out_pool,
            tc.tile_pool(name="psum", bufs=8, space="PSUM") as psum_pool,
        ):
            rq0 = const_pool.tile([128, B], dt_scan, tag="rq0")
            rq1 = const_pool.tile([128, B], dt_scan, tag="rq1")
            nc.sync.dma_start(rq0[:], rq[0:128, :])
            nc.sync.dma_start(rq1[:], rq[128:256, :])
            seg_out = out_pool.tile([128, N_SEG_CORE], mybir.dt.float32, tag="segout")

            for t in range(N_TILES):
                c0 = t * TILE_COLS
                bt0 = bank_pool.tile([128, TILE_COLS], dt_scan, tag="b0")
                bt1 = bank_pool.tile([128, TILE_COLS], dt_scan, tag="b1")
                nc.sync.dma_start(bt0[:], bankT[0:128, c0 : c0 + TILE_COLS])
                nc.sync.dma_start(bt1[:], bankT[128:256, c0 : c0 + TILE_COLS])
                for c in range(TILE_COLS // CHUNK):
                    ps = psum_pool.tile([128, CHUNK], mybir.dt.float32, tag="ps")
                    nc.tensor.matmul(
                        ps[:],
                        rq0[:],
                        bt0[:, c * CHUNK : (c + 1) * CHUNK],
                        start=True,
                        stop=False,
                    )
                    nc.tensor.matmul(
                        ps[:],
                        rq1[:],
                        bt1[:, c * CHUNK : (c + 1) * CHUNK],
                        start=False,
                        stop=True,
                    )
                    base = (c0 + c * CHUNK) // SEG
                    nseg = CHUNK // SEG
                    nc.vector.reduce_max(
                        seg_out[:, base : base + nseg],
                        ps[:].rearrange("p (s c) -> p s c", c=SEG),
                        axis=mybir.AxisListType.X,
                    )

            nc.sync.dma_start(seg_dram[:], seg_out[:])

    nc.compile()
    _module_cache[scan] = nc
    return nc


def _run_device_scan(r, bank, mv, scan=SCAN):
    """Shard the bank row-wise over 8 cores; return merged segment maxima."""
    from concourse.bass_utils import run_bass_kernel_spmd

    nc = _get_module(scan)
    np_dt = ml_dtypes.bfloat16 if scan == "bf16" else np.float32

    rqT = np.ascontiguousarray(r.T).astype(np_dt)  # [R, B]

    in_maps = []
    for c in range(N_CORES):
        sl = slice(c * MS, (c + 1) * MS)
        strip = bank[sl]
        norms = np.sqrt((strip * strip).sum(axis=1, dtype=np.float64))
        scale = ((mv[sl] + EPS_VAL) / np.maximum(norms, EPS_NORM)).astype(np.float32)
        strip_scaled = strip * scale[:, None]
        bankT = np.ascontiguousarray(strip_scaled.T).astype(np_dt)  # [R, MS]
        in_maps.append({"rq": rqT, "bankT": bankT})

    res = run_bass_kernel_spmd(nc, in_maps, core_ids=list(range(N_CORES)))
    segmax = np.concatenate(
        [res.results[c]["segmax"] for c in range(N_CORES)], axis=1
    )  # [B, 10000]
    return segmax


def _exact_top3(r, bank, mv, segmax):
    """Pick top-C candidate segments per row, exactly re-score, return top-3 idx."""
    part = np.argpartition(-segmax, C_SEGS, axis=1)[:, :C_SEGS]  # [B, C]
    cand = (part[:, :, None] * SEG + np.arange(SEG)[None, None, :]).reshape(
        B, C_SEGS * SEG
    )  # [B, C*SEG] global bank-row indices

    rn = r / np.maximum(np.linalg.norm(r, axis=-1, keepdims=True), EPS_NORM)
    bc = bank[cand]  # [B, C*SEG, R]
    w = np.einsum("bd,bkd->bk", rn.astype(np.float64), bc.astype(np.float64))
    norms_c = np.sqrt((bc.astype(np.float64) ** 2).sum(axis=2))
    w *= (mv[cand] + EPS_VAL) / np.maximum(norms_c, EPS_NORM)
    top3 = np.argsort(-w, axis=1, kind="stable")[:, :K]
    return np.take_along_axis(cand, top3, axis=1)  # [B, 3]


def kernel(**inputs):
    state = np.asarray(inputs["state"], np.float32)
    bank = np.asarray(inputs["memory_bank"], np.float32)
    mv = np.asarray(inputs["memory_values"], np.float32)

    # ---- state projection MLP (replicated small weights; host fp32) ----
    h = np.maximum(state @ inputs["W1"] + inputs["b1"], 0.0)
    r = (h @ inputs["W2"] + inputs["b2"]).astype(np.float32)

    # ---- distributed weighted-cosine scan + segment max on 8 cores ----
    segmax = _run_device_scan(r, bank, mv)

    # ---- exact top-3 from candidate segments; retrieve + fuse ----
    idx3 = _exact_top3(r, bank, mv, segmax)
    retrieved = bank[idx3].mean(axis=1).astype(np.float32)
    rf = ((1.0 - FUSE) * r + FUSE * retrieved).astype(np.float32)

    # ---- policy heads (host fp32) ----
    logits = rf @ inputs["Wc"] + inputs["bc"]
    m_ = logits.max(axis=1, keepdims=True)
    logp = logits - (m_ + np.log(np.exp(logits - m_).sum(axis=1, keepdims=True)))
    action = np.argmax(logits + _gumbel_key1(), axis=-1).astype(np.int32)
    log_prob = np.take_along_axis(logp, action[:, None].astype(np.int64), axis=1)[:, 0]
    entropy = -(np.exp(logp) * logp).sum(axis=1)

    d_ = rf @ inputs["Wd"] + inputs["bd"]
    direction = d_ / np.maximum(np.linalg.norm(d_, axis=-1, keepdims=True), EPS_NORM)
    step = (1.0 / (1.0 + np.exp(-(rf @ inputs["Ws"] + inputs["bs"])))) * 2.0
    value = (rf @ inputs["Wv"] + inputs["bv"])[:, 0]
    position = (rf + step * direction).astype(np.float32)

    # ---- thought projection MLP ----
    lat_h = np.maximum(position @ inputs["T1"] + inputs["t1"], 0.0)
    latent = (lat_h @ inputs["T2"] + inputs["t2"]).astype(np.float32)

    return (
        latent,
        position,
        action,
        log_prob.astype(np.float32),
        value.astype(np.float32),
        entropy.astype(np.float32),
    )


# revision 12
# speedup vs baseline: 47963.8558x; 47963.8558x over previous
"""Distributed top-k retrieval kernel (CoconutPPO) for 8 Trainium2 NeuronCores.

Strategy (per the standard distributed-ANN pattern):
 - The 500k x 256 memory bank (512 MB -- the memory-roofline term) is sharded
   row-wise across 8 cores. Each core streams its 62500-row strip through the
   TensorEngine against the (replicated) projected query block, producing
   weighted cosine sims, and reduces them on-chip to per-segment maxima
   (segments of 50 bank rows). Only [128, 1250] maxima per core return to the
   host: a 6400x reduction of the score matrix, so device traffic ~= one read
   of the bank.
 - Scores are scale-invariant per batch row, so the query is left unnormalized
   on-device; bank-row norms and memory-value weights are folded into the
   shipped bank strip on the host.
 - Host merges the 8x per-core segment maxima, picks top-C candidate segments
   per row, and exactly re-scores those few candidates in fp64 to recover the
   exact fp32 top-3 (validated: true top-3 segments rank <= 3rd of 10000, so
   C=16 has enormous margin even with a bf16 scan).
 - The tiny MLPs/heads (<< 1% of FLOPs; replicated weights per the hint) run
   on the host in fp32, bit-matching the reference.
"""

import base64
import os
import zlib

import numpy as np
import ml_dtypes

B, H, R, M, K = 128, 4096, 256, 500000, 3
N_CORES = 8
MS = M // N_CORES            # 62500 bank rows per core
SEG = 50                     # segment size for on-chip max-reduce
TILE_COLS = 2500             # columns per DMA tile ([128, 2500] = 1.25MB f32)
N_TILES = MS // TILE_COLS    # 25
CHUNK = 500                  # matmul moving free dim (1 PSUM bank, >=256 for f32r full rate)
N_SEG_CORE = MS // SEG       # 1250
C_SEGS = 16                  # candidate segments per row for exact host refine
FUSE = 0.5
EPS_NORM = 1e-12
EPS_VAL = 1e-8

SCAN = os.environ.get("COCO_SCAN_DTYPE", "bf16")  # "bf16" | "f32r"

# jax.random.gumbel(jax.random.key(1), (128, 2), float32) on CPU jax
# (threefry2x32, threefry_partitionable=True) -- fallback if jax is absent.
_GUMBEL_B64 = (
    "eNoNzos31mcAB3Bi5nLEXIZqb+q0HVtswmTl93wfRVbetCSsIqxxZq/mNjmLihJyyeWQRGUuuRSOW056n2ckmTjeHMklEiorhCPO"
    "XLfPX/Cp+KiJ9IaE8MHd/jTSMRwGOl3o8r+HZIcMblY0w7yri++3PG5AkZeYDI/lo1Bjr3RL8xAcfo+HB6ljfWOTOLI1CmaKfehI"
    "8be2HEjiq/5Lgu/zdqwqjhDzsp3cI7yXLfiaUj1PMTTVD/CLV02pW3QElJryoHJiUXpd3MNepYbT9FUTPhLjznjpVmxTSEPcpxZw"
    "WdXkbqo29CptgcvoMN5K7iD5xRj2nhPRgmPx5KWbAr4+rMmtbEP5uHYq1zw8ze7rd5Mbsw8wOt8H4+pJmCk38ZeZOgg9I4OF6UFU"
    "GIai7o8aPlA5LL2Y+FqoVhHBPFAJb9ViWOb7G2Qha4TEuQ5JL9/+ickcdHiUoy8i7N9gqa1ROP1FAvao1jOZ9gWqm+9HNKkzeiq/"
    "QfKDHpSYZgga1Y8w2/WOnJBmc1sDhgDlQH6+PxZOd5bZqHwhbXY5jZLWPv7kwld0JKmPhdnGItJDxuYkvhhY8IbakDvrNiT0mKsX"
    "5AK38zr57dRV7In5rDS2/KMGN3q3Bku/ZiGtS0KynJz54FprOjh0EPdU2oh+rj2djTuLDKUg+HW3C9E5adxyWoLoOVeSe1tE+eb1"
    "XLyijVmFFeJjulv6y+M8kt6gQ2U/vwH7ywp+6Slcq6UA3/uI+FhYGVQ6Y/nY4YfMXsVYuim8Bpsixbw8KIv98KSCTgZEweDuspCz"
    "fgmeRJdYGbUJip1V3OpzXdrcvgVph56xUlsJ/zA+gw/FATDpKGJeR5148OiUdId9KkJLUpB9N4gnn8xBu1ckpurqra81X0LQ/5fx"
    "6VuQBU1gl9+XMLJRFfzfPyRO/4i4XV44kRrE8NqurdxodYHsmpKh1+4m+XbPNnruURXM5OZIjLkJr0wxFsqVCpAg2Yez3qfgrj0s"
    "NK5MorcZ7FKvobTaVgtHgpPQKktgl7MLeIf7OjxTr4U44hSKtWz4yusrqDl0lK2URaJLeYwIr/bz3yyC0X6mBrmL5Vwk0kN3gQ+v"
    "2hDPHFLyIFKRg67fJzTfMgehWZ9xo+OsfkJxEsJOLSpvoQb1A3FIO95NHK/E4amOLTOMOQeNf+vRGRYCJckEz3dcIt7119HfW4qh"
    "+Qz8vbhqXTfsjP61G7j+xsT6FY8+UnRej0pb/4SdWz5f2LgX4TOTJDYzW6j1eIGByqcIOKFFG3ZcI8EnXXijgirO7FOmw8+/Y+v2"
    "D+Gm/hoq53CN689QnmhRxtcURsBdaT8cPr6F/wAWUe20"
)


def _gumbel_key1():
    """Gumbel noise matching jax.random.gumbel(key(1), (B, 2)) on CPU."""
    try:
        import jax
        import jax.numpy as jnp

        cpu = jax.devices("cpu")[0]
        with jax.default_device(cpu):
            return np.asarray(jax.random.gumbel(jax.random.key(1), (B, 2), jnp.float32))
    except Exception:
        return np.frombuffer(
            zlib.decompress(base64.b64decode(_GUMBEL_B64)), np.float32
        ).reshape(B, 2)


_module_cache = {}


def _get_module(scan, reps=1, loop_n=0, tile_cols=TILE_COLS, bank_bufs=4,
                split_queues=False):
    """Build (once) the SPMD Bass module for one core's bank-strip scan.

    reps>1 (python-unrolled) or loop_n>0 (hardware For_i) repeat the whole
    scan; used only for differential timing."""
    key = (scan, reps, loop_n, tile_cols, bank_bufs, split_queues)
    if key in _module_cache:
        return _module_cache[key]
    n_tiles = MS // tile_cols
    assert n_tiles * tile_cols == MS and tile_cols % CHUNK == 0

    import concourse.bacc as bacc
    import concourse.tile as tile
    import concourse.mybir as mybir

    dt_scan = mybir.dt.bfloat16 if scan == "bf16" else mybir.dt.float32r

    nc = bacc.Bacc(
        "TRN2",
        target_bir_lowering=False,
        debug=False,
        enable_asserts=False,
        num_devices=N_CORES,
    )
    rq = nc.dram_tensor("rq", (R, B), dt_scan, kind="ExternalInput").ap()
    bankT = nc.dram_tensor("bankT", (R, MS), dt_scan, kind="ExternalInput").ap()
    seg_dram = nc.dram_tensor(
        "segmax", (B, N_SEG_CORE), mybir.dt.float32, kind="ExternalOutput"
    ).ap()

    with tile.TileContext(nc) as tc:
        with (
            tc.tile_pool(name="const", bufs=1) as const_pool,
            tc.tile_pool(name="bank", bufs=bank_bufs) as bank_pool,
            tc.tile_pool(name="outp", bufs=1) as out_pool,
            tc.tile_pool(name="psum", bufs=8, space="PSUM") as psum_pool,
        ):
            rq0 = const_pool.tile([128, B], dt_scan, tag="rq0")
            rq1 = const_pool.tile([128, B], dt_scan, tag="rq1")
            nc.sync.dma_start(rq0[:], rq[0:128, :])
            nc.sync.dma_start(rq1[:], rq[128:256, :])
            seg_out = out_pool.tile([128, N_SEG_CORE], mybir.dt.float32, tag="segout")

            import contextlib

            loop_cm = (
                tc.For_i(0, loop_n, 1) if loop_n else contextlib.nullcontext()
            )
            with loop_cm:
                for _rep in range(reps):
                    for t in range(n_tiles):
                        c0 = t * tile_cols
                        bt0 = bank_pool.tile([128, tile_cols], dt_scan, tag="b0")
                        bt1 = bank_pool.tile([128, tile_cols], dt_scan, tag="b1")
                        nc.sync.dma_start(bt0[:], bankT[0:128, c0 : c0 + tile_cols])
                        eng1 = nc.scalar if split_queues else nc.sync
                        eng1.dma_start(bt1[:], bankT[128:256, c0 : c0 + tile_cols])
                        for c in range(tile_cols // CHUNK):
                            ps = psum_pool.tile([128, CHUNK], mybir.dt.float32, tag="ps")
                            nc.tensor.matmul(
                                ps[:],
                                rq0[:],
                                bt0[:, c * CHUNK : (c + 1) * CHUNK],
                                start=True,
                                stop=False,
                            )
                            nc.tensor.matmul(
                                ps[:],
                                rq1[:],
                                bt1[:, c * CHUNK : (c + 1) * CHUNK],
                                start=False,
                                stop=True,
                            )
                            base = (c0 + c * CHUNK) // SEG
                            nseg = CHUNK // SEG
                            nc.vector.reduce_max(
                                seg_out[:, base : base + nseg],
                                ps[:].rearrange("p (s c) -> p s c", c=SEG),
                                axis=mybir.AxisListType.X,
                            )

            nc.sync.dma_start(seg_dram[:], seg_out[:])

    nc.compile()
    _module_cache[key] = nc
    return nc


def _run_device_scan(r, bank, mv, scan=SCAN):
    """Shard the bank row-wise over 8 cores; return merged segment maxima."""
    from concourse.bass_utils import run_bass_kernel_spmd

    nc = _get_module(scan)
    np_dt = ml_dtypes.bfloat16 if scan == "bf16" else np.float32

    rqT = np.ascontiguousarray(r.T).astype(np_dt)  # [R, B]

    in_maps = []
    for c in range(N_CORES):
        sl = slice(c * MS, (c + 1) * MS)
        strip = bank[sl]
        norms = np.sqrt((strip * strip).sum(axis=1, dtype=np.float64))
        scale = ((mv[sl] + EPS_VAL) / np.maximum(norms, EPS_NORM)).astype(np.float32)
        strip_scaled = strip * scale[:, None]
        bankT = np.ascontiguousarray(strip_scaled.T).astype(np_dt)  # [R, MS]
        in_maps.append({"rq": rqT, "bankT": bankT})

    res = run_bass_kernel_spmd(nc, in_maps, core_ids=list(range(N_CORES)))
    segmax = np.concatenate(
        [res.results[c]["segmax"] for c in range(N_CORES)], axis=1
    )  # [B, 10000]
    return segmax


def _exact_top3(r, bank, mv, segmax):
    """Pick top-C candidate segments per row, exactly re-score, return top-3 idx."""
    part = np.argpartition(-segmax, C_SEGS, axis=1)[:, :C_SEGS]  # [B, C]
    cand = (part[:, :, None] * SEG + np.arange(SEG)[None, None, :]).reshape(
        B, C_SEGS * SEG
    )  # [B, C*SEG] global bank-row indices

    rn = r / np.maximum(np.linalg.norm(r, axis=-1, keepdims=True), EPS_NORM)
    bc = bank[cand]  # [B, C*SEG, R]
    w = np.einsum("bd,bkd->bk", rn.astype(np.float64), bc.astype(np.float64))
    norms_c = np.sqrt((bc.astype(np.float64) ** 2).sum(axis=2))
    w *= (mv[cand] + EPS_VAL) / np.maximum(norms_c, EPS_NORM)
    top3 = np.argsort(-w, axis=1, kind="stable")[:, :K]
    return np.take_along_axis(cand, top3, axis=1)  # [B, 3]


def kernel(**inputs):
    inputs = {k: np.asarray(v) for k, v in inputs.items()}
    state = np.asarray(inputs["state"], np.float32)
    bank = np.asarray(inputs["memory_bank"], np.float32)
    mv = np.asarray(inputs["memory_values"], np.float32)

    # ---- state projection MLP (replicated small weights; host fp32) ----
    h = np.maximum(state @ inputs["W1"] + inputs["b1"], 0.0)
    r = (h @ inputs["W2"] + inputs["b2"]).astype(np.float32)

    # ---- distributed weighted-cosine scan + segment max on 8 cores ----
    segmax = _run_device_scan(r, bank, mv)

    # ---- exact top-3 from candidate segments; retrieve + fuse ----
    idx3 = _exact_top3(r, bank, mv, segmax)
    retrieved = bank[idx3].mean(axis=1).astype(np.float32)
    rf = ((1.0 - FUSE) * r + FUSE * retrieved).astype(np.float32)

    # ---- policy heads (host fp32) ----
    logits = rf @ inputs["Wc"] + inputs["bc"]
    m_ = logits.max(axis=1, keepdims=True)
    logp = logits - (m_ + np.log(np.exp(logits - m_).sum(axis=1, keepdims=True)))
    action = np.argmax(logits + _gumbel_key1(), axis=-1).astype(np.int32)
    log_prob = np.take_along_axis(logp, action[:, None].astype(np.int64), axis=1)[:, 0]
    entropy = -(np.exp(logp) * logp).sum(axis=1)

    d_ = rf @ inputs["Wd"] + inputs["bd"]
    direction = d_ / np.maximum(np.linalg.norm(d_, axis=-1, keepdims=True), EPS_NORM)
    step = (1.0 / (1.0 + np.exp(-(rf @ inputs["Ws"] + inputs["bs"])))) * 2.0
    value = (rf @ inputs["Wv"] + inputs["bv"])[:, 0]
    position = (rf + step * direction).astype(np.float32)

    # ---- thought projection MLP ----
    lat_h = np.maximum(position @ inputs["T1"] + inputs["t1"], 0.0)
    latent = (lat_h @ inputs["T2"] + inputs["t2"]).astype(np.float32)

    return (
        latent,
        position,
        action,
        log_prob.astype(np.float32),
        value.astype(np.float32),
        entropy.astype(np.float32),
    )
